# revision 9
# baseline (speedup 1.0000x reference)
"""AttentionDecoder Trainium2 kernel — 8-core SPMD, v2.

Strategy:
  - Data-parallel recurrence: core c owns batch slice [8c, 8c+8).
    LSTM+attention runs fully on-device in a single fused loop.
  - Hidden states kept in ONE bf16 transposed history buffer (t-major),
    with an extra t=-1 slot holding h0 (computed host-side along with c0
    from the tiny NL2-weight init projections).
  - Chunked AllGather: hs for t<16 is gathered at step 16 and the first
    half of the vocab projection is interleaved under steps 19..31; the
    second half runs as a short tail after step 31.
  - Vocab-parallel output projection: core c computes
    preds[:, :, 4000c:4000c+4000]; host concatenates the 8 vocab shards.
  - Engine balance: esum-add + alpha-broadcast + ctx-mult on GpSimd,
    reductions/copies on Vector, activations on Scalar, gates reordered
    to [i,f,o,g] so one sigmoid covers 3 gates.
"""

import os
import sys

sys.path.insert(0, "/opt/trn_rl_repo")

import ml_dtypes
import numpy as np

import concourse.bass as bass
from concourse import bacc
import concourse.mybir as mybir
import concourse.tile as tile
from concourse.bass_utils import run_bass_kernel_spmd
from concourse.masks import make_identity

# problem shapes (hardcoded per harness contract)
B, S, H, E, V, NL2, T = 64, 64, 512, 256, 32000, 4, 32
NCORES = 8
BL = B // NCORES  # 8 examples per core
VL = V // NCORES  # 4000 vocab rows per core
EPS = 1e-5
BS = BL * S  # 512 rows of encoder per core
TB = T * BL  # 256 (t, b) rows per core
G4 = 4 * H  # 2048 gate dim
BT = B * T  # 2048 gathered rows
KT = H // 128  # 4 partition tiles for the 512 hidden dim
KSTR = (T + 1) * BL  # 264: per-k stride in the hs history (slot 0 = h0)
TCH = T // 2  # 16 steps per projection chunk
CCH = TCH * BL  # 128 hs cols per chunk

F32 = mybir.dt.float32
F32R = mybir.dt.float32r
BF16 = mybir.dt.bfloat16
I32 = mybir.dt.int32
AF = mybir.ActivationFunctionType
ALU = mybir.AluOpType

bf16 = ml_dtypes.bfloat16


def _bc_free(ap, n):
    """Append a step-0 free dim of size n (broadcast along a new inner axis)."""
    return bass.AP(tensor=ap.tensor, offset=ap.offset, ap=[*ap.ap, [0, n]])


def _bc_col(ap, n):
    """[P, 1] column -> [P, n] broadcast (replace free dim with step-0)."""
    return bass.AP(tensor=ap.tensor, offset=ap.offset, ap=[ap.ap[0], [0, n]])


def build_nc():
    nc = bacc.Bacc()

    # ---------------- DRAM I/O ----------------
    d_enc = nc.dram_tensor("enc", [BS, H], F32, kind="ExternalInput")
    d_h0T = nc.dram_tensor("h0T", [H, BL], BF16, kind="ExternalInput")
    d_c0 = nc.dram_tensor("c0", [BL, H], F32, kind="ExternalInput")
    d_emb = nc.dram_tensor("emb", [V, E], F32, kind="ExternalInput")
    d_tgt = nc.dram_tensor("tgt", [TB, 1], I32, kind="ExternalInput")
    d_kwT = nc.dram_tensor("kwT", [H, H], BF16, kind="ExternalInput")
    d_qwT = nc.dram_tensor("qwT", [H, H], BF16, kind="ExternalInput")
    d_ewT = nc.dram_tensor("ewT", [H, 1], BF16, kind="ExternalInput")
    d_qadd = nc.dram_tensor("qadd", [128, KT], F32, kind="ExternalInput")
    d_wcT = nc.dram_tensor("wcT", [H, G4], BF16, kind="ExternalInput")
    d_whT = nc.dram_tensor("whT", [H, G4], BF16, kind="ExternalInput")
    d_xwT = nc.dram_tensor("xwT", [2 * 128 + 1, G4], BF16, kind="ExternalInput")
    d_owT = nc.dram_tensor("owT", [H, VL], BF16, kind="ExternalInput")
    d_ob = nc.dram_tensor("ob", [128, VL], BF16, kind="ExternalInput")
    d_out = nc.dram_tensor("out", [B, T, VL], F32, kind="ExternalOutput")

    # internal DRAM for the two chunked collectives (+ warmup)
    d_ccwin = nc.dram_tensor("ccwin", [1, 64], BF16)
    d_ccwout = nc.dram_tensor("ccwout", [NCORES, 64], BF16, addr_space="Shared")
    d_ccin = [nc.dram_tensor(f"ccin{c}", [H, CCH], BF16) for c in range(2)]
    d_ccout = [
        nc.dram_tensor(f"ccout{c}", [NCORES * H, CCH], BF16, addr_space="Shared")
        for c in range(2)
    ]

    with tile.TileContext(nc) as tc:
        with (
            tc.tile_pool(name="persist", bufs=1) as P_per,
            tc.tile_pool(name="cell", bufs=2) as P_cell,
            tc.tile_pool(name="projout", bufs=4) as P_po,
        ):
            # ---------- persistent SBUF ----------
            id128 = P_per.tile([128, 128], F32, name="id128")
            make_identity(nc, id128[:, :])
            id8 = P_per.tile([8, 8], F32, name="id8")
            make_identity(nc, id8[:, :])
            eps_t = P_per.tile([128, 1], F32, name="eps")
            nc.vector.memset(eps_t[:, :], EPS)

            # bf16 transposed hidden history: col = k*KSTR + (t+1)*BL + b
            hs_Tb = P_per.tile([128, KT * KSTR], BF16, name="hsTb")
            c_st = P_per.tile([BL, H], F32, name="c_state")
            nc.sync.dma_start(c_st[:, :], d_c0[:, :])
            for k in range(KT):
                nc.sync.dma_start(
                    hs_Tb[:, k * KSTR : k * KSTR + BL],
                    d_h0T[128 * k : 128 * (k + 1), :],
                )

            # ---------- resident weights ----------
            kwT = [P_per.tile([128, H], BF16, name=f"kwT{k}") for k in range(KT)]
            qwT = [P_per.tile([128, H], BF16, name=f"qwT{k}") for k in range(KT)]
            ewT = [P_per.tile([128, 1], BF16, name=f"ewT{k}") for k in range(KT)]
            qadd4 = P_per.tile([128, KT], F32, name="qadd4")
            nc.sync.dma_start(qadd4[:, :], d_qadd[:, :])
            wcT = [P_per.tile([128, G4], BF16, name=f"wcT{k}") for k in range(KT)]
            whT = [P_per.tile([128, G4], BF16, name=f"whT{k}") for k in range(KT)]
            for k in range(KT):
                nc.sync.dma_start(kwT[k][:, :], d_kwT[128 * k : 128 * (k + 1), :])
                nc.sync.dma_start(qwT[k][:, :], d_qwT[128 * k : 128 * (k + 1), :])
                nc.sync.dma_start(ewT[k][:, :], d_ewT[128 * k : 128 * (k + 1), :])
                nc.sync.dma_start(wcT[k][:, :], d_wcT[128 * k : 128 * (k + 1), :])
                nc.sync.dma_start(whT[k][:, :], d_whT[128 * k : 128 * (k + 1), :])
            owT = [P_per.tile([128, VL], BF16, name=f"owT{k}") for k in range(KT)]
            for k in range(KT):
                nc.sync.dma_start(owT[k][:, :], d_owT[128 * k : 128 * (k + 1), :])
            ob_bc = P_per.tile([128, VL], BF16, name="ob_bc")
            nc.sync.dma_start(ob_bc[:, :], d_ob[:, :])

            ones_col = P_per.tile([1, 128], BF16, name="ones_col")
            nc.vector.memset(ones_col[:, :], 1.0)
            nc.sync.dma_start(d_ccwin[:, :], ones_col[:, 0:64])
            nc.gpsimd.collective_compute(
                "AllGather",
                ALU.bypass,
                replica_groups=[list(range(NCORES))],
                ins=[d_ccwin[:, :]],
                outs=[d_ccwout[:, :]],
            )
            enc_Tb = [P_per.tile([128, BS], BF16, name=f"encT{k}") for k in range(KT)]
            pk_Tb = [P_per.tile([128, BS], BF16, name=f"pkT{g}") for g in range(KT)]
            x_gates = [P_per.tile([128, G4], BF16, name=f"xg{m}") for m in range(2)]
            hs_all = [P_per.tile([128, NCORES * CCH], BF16, name=f"hsall{k}")
                      for k in range(KT)]

            # ========== precompute (scoped pools; freed before recurrence) ====
            with (
                tc.tile_pool(name="pre", bufs=1) as P_pre,
                tc.tile_pool(name="prew", bufs=2) as P_pw,
                tc.tile_pool(name="psPre", bufs=3, space="PSUM") as PS_pre,
                tc.tile_pool(name="psPreMM", bufs=2, space="PSUM") as PS_pmm,
            ):
                xwT = [P_pre.tile([128, G4], BF16, name=f"xwT{k}") for k in range(2)]
                xwTb = P_pre.tile([1, G4], BF16, name="xwTbias")
                for k in range(2):
                    nc.sync.dma_start(xwT[k][:, :], d_xwT[128 * k : 128 * (k + 1), :])
                nc.sync.dma_start(xwTb[:, :], d_xwT[2 * 128 : 2 * 128 + 1, :])

                # ----- encoder LN (natural layout) + transpose -----
                enc_ln = [P_pre.tile([128, H], F32, name=f"encln{i}") for i in range(4)]
                for i in range(BS // 128):
                    x_t = P_pw.tile([128, H], F32, name="enc_in")
                    nc.sync.dma_start(x_t[:, :], d_enc[128 * i : 128 * (i + 1), :])
                    stats = P_pw.tile([128, 6], F32, name="enc_st")
                    mv = P_pw.tile([128, 2], F32, name="enc_mv")
                    nc.vector.bn_stats(out=stats[:, :], in_=x_t[:, :])
                    nc.vector.bn_aggr(out=mv[:, :], in_=stats[:, :])
                    nc.scalar.activation(
                        out=mv[:, 1:2], in_=mv[:, 1:2], func=AF.Sqrt, bias=eps_t[:, :]
                    )
                    nc.vector.reciprocal(out=mv[:, 1:2], in_=mv[:, 1:2])
                    nc.vector.tensor_scalar(
                        out=enc_ln[i][:, :],
                        in0=x_t[:, :],
                        scalar1=mv[:, 0:1],
                        scalar2=mv[:, 1:2],
                        op0=ALU.subtract,
                        op1=ALU.mult,
                    )
                for i in range(4):
                    for j in range(4):
                        pt = PS_pre.tile([128, 128], F32, name="tpose")
                        nc.tensor.transpose(
                            out=pt[:, :],
                            in_=enc_ln[i][:, 128 * j : 128 * (j + 1)],
                            identity=id128[:, :],
                        )
                        nc.vector.tensor_copy(
                            enc_Tb[j][:, 128 * i : 128 * (i + 1)], pt[:, :]
                        )

                # ----- projected keys pk_T (bf16) -----
                for g in range(KT):
                    pp = PS_pmm.tile([128, BS], F32, name="pk_ps")
                    for k in range(KT):
                        nc.tensor.matmul(
                            pp[:, :],
                            lhsT=kwT[k][:, 128 * g : 128 * (g + 1)],
                            rhs=enc_Tb[k][:, :],
                            start=(k == 0),
                            stop=(k == KT - 1),
                        )
                    nc.vector.tensor_copy(pk_Tb[g][:, :], pp[:, :])

                # ----- embedding gather + LN + transpose -----
                xe_ln = [P_pre.tile([128, E], F32, name=f"xeln{i}") for i in range(2)]
                for i in range(2):
                    tgt_sb = P_pw.tile([128, 1], I32, name="tgt")
                    nc.sync.dma_start(tgt_sb[:, :], d_tgt[128 * i : 128 * (i + 1), :])
                    xg = P_pw.tile([128, E], F32, name="xemb")
                    nc.gpsimd.indirect_dma_start(
                        out=xg[:, :],
                        out_offset=None,
                        in_=d_emb[:, :],
                        in_offset=bass.IndirectOffsetOnAxis(ap=tgt_sb[:, 0:1], axis=0),
                    )
                    stats = P_pw.tile([128, 6], F32, name="xe_st")
                    mv = P_pw.tile([128, 2], F32, name="xe_mv")
                    nc.vector.bn_stats(out=stats[:, :], in_=xg[:, :])
                    nc.vector.bn_aggr(out=mv[:, :], in_=stats[:, :])
                    nc.scalar.activation(
                        out=mv[:, 1:2], in_=mv[:, 1:2], func=AF.Sqrt, bias=eps_t[:, :]
                    )
                    nc.vector.reciprocal(out=mv[:, 1:2], in_=mv[:, 1:2])
                    nc.vector.tensor_scalar(
                        out=xe_ln[i][:, :],
                        in0=xg[:, :],
                        scalar1=mv[:, 0:1],
                        scalar2=mv[:, 1:2],
                        op0=ALU.subtract,
                        op1=ALU.mult,
                    )
                xlnT = [P_pre.tile([128, TB], BF16, name=f"xlnT{k}") for k in range(2)]
                for i in range(2):
                    for j in range(2):
                        pt = PS_pre.tile([128, 128], F32, name="tpose")
                        nc.tensor.transpose(
                            out=pt[:, :],
                            in_=xe_ln[i][:, 128 * j : 128 * (j + 1)],
                            identity=id128[:, :],
                        )
                        nc.vector.tensor_copy(
                            xlnT[j][:, 128 * i : 128 * (i + 1)], pt[:, :]
                        )
                ones_row = P_pre.tile([1, 128], BF16, name="ones")
                nc.vector.memset(ones_row[:, :], 1.0)

                # ----- x_gates = LN(emb[tgt]) @ w_x.T + b (iofg order) -----
                for m in range(2):
                    for b4 in range(4):
                        pp = PS_pmm.tile([128, 512], F32, name="xg_ps")
                        for k in range(2):
                            nc.tensor.matmul(
                                pp[:, :],
                                lhsT=xlnT[k][:, 128 * m : 128 * (m + 1)],
                                rhs=xwT[k][:, 512 * b4 : 512 * (b4 + 1)],
                                start=(k == 0),
                                stop=False,
                            )
                        nc.tensor.matmul(
                            pp[:, :],
                            lhsT=ones_row[:, :],
                            rhs=xwTb[:, 512 * b4 : 512 * (b4 + 1)],
                            start=False,
                            stop=True,
                        )
                        nc.vector.tensor_copy(
                            x_gates[m][:, 512 * b4 : 512 * (b4 + 1)], pp[:, :]
                        )

            # ============== recurrence + interleaved projection ==============
            with (
                tc.tile_pool(name="psSmall", bufs=3, space="PSUM") as PS_s,
                tc.tile_pool(name="psG", bufs=3, space="PSUM") as PS_g,
                tc.tile_pool(name="psProj", bufs=2, space="PSUM") as PS_p,
            ):
                NV = VL // 500  # 8 vocab chunks of 500

                def h_col(t):
                    # hs col base for h_{t} (slot t+1); k-tile k at + k*KSTR
                    return (t + 1) * BL

                def emit_proj_group(chunk, mt, vc):
                    pp = PS_p.tile([128, 500], F32, name="proj_ps")
                    for k in range(KT):
                        nc.tensor.matmul(
                            pp[:, :],
                            lhsT=hs_all[k][:, 128 * mt : 128 * (mt + 1)],
                            rhs=owT[k][:, 500 * vc : 500 * (vc + 1)],
                            start=(k == 0),
                            stop=(k == KT - 1),
                        )
                    ob_t = P_po.tile([128, 500], F32, name="proj_out")
                    nc.vector.tensor_tensor(
                        out=ob_t[:, :],
                        in0=pp[:, :],
                        in1=ob_bc[:, 500 * vc : 500 * (vc + 1)],
                        op=ALU.add,
                    )
                    # psum rows are (t, b) t-major for core mt's batch rows
                    dst = d_out[
                        BL * mt : BL * (mt + 1),
                        TCH * chunk : TCH * (chunk + 1),
                        500 * vc : 500 * (vc + 1),
                    ].rearrange("b t v -> t b v")
                    nc.sync.dma_start(dst, ob_t[:, :])

                def emit_gather(chunk):
                    base = CCH * chunk
                    for k in range(KT):
                        nc.sync.dma_start(
                            d_ccin[chunk][128 * k : 128 * (k + 1), :],
                            hs_Tb[:, k * KSTR + BL + base : k * KSTR + BL + base + CCH],
                        )
                    nc.gpsimd.collective_compute(
                        "AllGather",
                        ALU.bypass,
                        replica_groups=[list(range(NCORES))],
                        ins=[d_ccin[chunk][:, :]],
                        outs=[d_ccout[chunk][:, :]],
                    )
                    for r in range(NCORES):
                        for k in range(KT):
                            nc.sync.dma_start(
                                hs_all[k][:, CCH * r : CCH * (r + 1)],
                                d_ccout[chunk][
                                    H * r + 128 * k : H * r + 128 * (k + 1), :
                                ],
                            )

                # interleave schedule: (step -> list of (chunk, mt, vc))
                proj_sched = {}
                groups0 = [(0, mt, vc) for mt in range(NCORES) for vc in range(NV)]
                PROJ_START = 19
                per_step = -(-len(groups0) // (T - PROJ_START))  # ceil
                for i, grp in enumerate(groups0):
                    proj_sched.setdefault(PROJ_START + i // per_step, []).append(grp)

                for t in range(T):
                    hb = h_col(t - 1)

                    # --- q = qw.T @ h (+qadd): one PSUM bank, 4 copies ---
                    q_ps = PS_s.tile([128, KT * BL], F32, name="q_ps", tag="sm")
                    q_Tb = P_cell.tile([128, KT * BL], BF16, name="qT")
                    for g in range(KT):
                        for k in range(KT):
                            nc.tensor.matmul(
                                q_ps[:, BL * g : BL * (g + 1)],
                                lhsT=qwT[k][:, 128 * g : 128 * (g + 1)],
                                rhs=hs_Tb[:, k * KSTR + hb : k * KSTR + hb + BL],
                                start=(k == 0),
                                stop=(k == KT - 1),
                            )
                        nc.vector.tensor_tensor(
                            out=q_Tb[:, BL * g : BL * (g + 1)],
                            in0=q_ps[:, BL * g : BL * (g + 1)],
                            in1=_bc_col(qadd4[:, g : g + 1], BL),
                            op=ALU.add,
                        )

                    # --- gates h-part early (fills tensor while attention runs)
                    pg = []
                    for b4 in range(4):
                        pgb = PS_g.tile([128, 512], F32, name="g_ps", tag="gps")
                        pg.append(pgb)
                        for k in range(KT):
                            nc.tensor.matmul(
                                pgb[0:BL, :],
                                lhsT=hs_Tb[:, k * KSTR + hb : k * KSTR + hb + BL],
                                rhs=whT[k][:, 512 * b4 : 512 * (b4 + 1)],
                                start=(k == 0),
                                stop=False,
                            )

                    # --- E = tanh(q + pk); energies = e_w . E ---
                    e_ps = PS_s.tile([1, BS], F32, name="e_ps", tag="sm")
                    esum = [
                        P_cell.tile([128, BS], BF16, name=f"esum{g}", bufs=1) for g in range(KT)
                    ]
                    eeng = nc.vector if TCH <= t < TCH + 4 else nc.gpsimd
                    for g in range(KT):
                        eeng.tensor_tensor(
                            out=esum[g][:, :].rearrange("p (b s) -> p b s", s=S),
                            in0=pk_Tb[g][:, :].rearrange("p (b s) -> p b s", s=S),
                            in1=_bc_free(q_Tb[:, BL * g : BL * (g + 1)], S),
                            op=ALU.add,
                        )
                        nc.scalar.activation(
                            out=esum[g][:, :], in_=esum[g][:, :], func=AF.Tanh
                        )
                        nc.tensor.matmul(
                            e_ps[:, :],
                            lhsT=ewT[g][:, :],
                            rhs=esum[g][:, :],
                            start=(g == 0),
                            stop=(g == KT - 1),
                        )

                    # --- softmax over s, normalization deferred to ctx ---
                    # exp (unnormalized); normalization deferred to ctx scale
                    alpha = P_cell.tile([1, BS], BF16, name="alpha")
                    nc.scalar.activation(out=alpha[:, :], in_=e_ps[:, :], func=AF.Exp)
                    # broadcast exp to 128 partitions via rank-1 matmul (on-chain)
                    abc_ps = PS_s.tile([128, BS], F32, name="abc_ps", tag="sm")
                    nc.tensor.matmul(
                        abc_ps[:, :],
                        lhsT=ones_col[:, :],
                        rhs=alpha[:, :],
                        start=True,
                        stop=True,
                    )
                    alpha_bc = P_cell.tile([128, BS], BF16, name="alpha_bc", bufs=1)
                    nc.vector.tensor_copy(alpha_bc[:, :], abc_ps[:, :])
                    # off-chain: row-sum, reciprocal, broadcast to partitions
                    ssum = P_cell.tile([1, BL], F32, name="ssum")
                    nc.vector.tensor_reduce(
                        out=ssum[:, :],
                        in_=alpha[:, :].rearrange("p (b s) -> p b s", s=S),
                        axis=mybir.AxisListType.X,
                        op=ALU.add,
                    )
                    nc.vector.reciprocal(out=ssum[:, :], in_=ssum[:, :])
                    rsum_bc = P_cell.tile([128, BL], F32, name="rsum_bc")
                    nc.gpsimd.partition_broadcast(rsum_bc[:, :], ssum[:, :])

                    # --- context_T[g, b] = (sum_s exp * enc_T) / ssum ---
                    ctxf = P_cell.tile([128, KT * BL], F32, name="ctxf")
                    ctx32 = P_cell.tile([128, KT * BL], BF16, name="ctx32")
                    veng = nc.vector if TCH <= t < TCH + 4 else nc.gpsimd
                    for g in range(KT):
                        prod = P_cell.tile(
                            [128, BS], BF16, name="ctx_prod", tag="prod", bufs=2
                        )
                        veng.tensor_tensor(
                            out=prod[:, :],
                            in0=enc_Tb[g][:, :],
                            in1=alpha_bc[:, :],
                            op=ALU.mult,
                        )
                        nc.vector.tensor_reduce(
                            out=ctxf[:, BL * g : BL * (g + 1)],
                            in_=prod[:, :].rearrange("p (b s) -> p b s", s=S),
                            axis=mybir.AxisListType.X,
                            op=ALU.add,
                        )

                    nc.vector.tensor_tensor(
                        out=ctx32[:, :].rearrange("p (g b) -> p g b", g=KT),
                        in0=ctxf[:, :].rearrange("p (g b) -> p g b", g=KT),
                        in1=bass.AP(
                            tensor=rsum_bc.tensor,
                            offset=rsum_bc.offset,
                            ap=[rsum_bc.ap[0], [0, KT], [1, BL]],
                        ),
                        op=ALU.mult,
                    )

                    # --- gates ctx-part + x_gates add ---
                    gates = P_cell.tile([BL, G4], F32, name="gates", bufs=1)
                    xg_t = P_cell.tile([BL, G4], BF16, name="xg_t", bufs=3)
                    xrow = BL * (t % TCH)
                    nc.sync.dma_start(
                        xg_t[:, :], x_gates[t // TCH][xrow : xrow + BL, :]
                    )
                    for b4 in range(4):
                        for k in range(KT):
                            nc.tensor.matmul(
                                pg[b4][0:BL, :],
                                lhsT=ctx32[:, BL * k : BL * (k + 1)],
                                rhs=wcT[k][:, 512 * b4 : 512 * (b4 + 1)],
                                start=False,
                                stop=(k == KT - 1),
                            )
                        nc.vector.tensor_tensor(
                            out=gates[:, 512 * b4 : 512 * (b4 + 1)],
                            in0=pg[b4][0:BL, :],
                            in1=xg_t[:, 512 * b4 : 512 * (b4 + 1)],
                            op=ALU.add,
                        )

                    # --- LSTM cell (iofg order: g0=i, g1=f, g2=o, g3=g~) ---
                    g_i = gates[:, 0:H]
                    g_f = gates[:, H : 2 * H]
                    g_o = gates[:, 2 * H : 3 * H]
                    g_g = gates[:, 3 * H : 4 * H]
                    nc.scalar.activation(
                        out=gates[:, 0 : 3 * H], in_=gates[:, 0 : 3 * H],
                        func=AF.Sigmoid,
                    )
                    nc.scalar.activation(out=g_g, in_=g_g, func=AF.Tanh)
                    nc.vector.tensor_mul(g_f, g_f, c_st[:, :])  # f*c
                    nc.vector.tensor_mul(g_i, g_i, g_g)  # i*g~
                    nc.vector.tensor_add(c_st[:, :], g_i, g_f)  # c2
                    nc.scalar.activation(out=g_g, in_=c_st[:, :], func=AF.Tanh)
                    h2 = P_cell.tile([BL, H], F32, name="h2", bufs=1)
                    nc.vector.tensor_mul(h2[:, :], g_o, g_g)

                    # --- transpose h2 into history (one bank, one copy) ---
                    htp = PS_s.tile([128, KT * BL], F32, name="htp", tag="sm")
                    for k in range(KT):
                        nc.tensor.transpose(
                            out=htp[:, BL * k : BL * (k + 1)],
                            in_=h2[:, 128 * k : 128 * (k + 1)],
                            identity=id8[:, :],
                        )
                    dst = hs_Tb[:, :].rearrange(
                        "p (k t b) -> p k t b", k=KT, b=BL
                    )[:, :, t + 1, :]
                    nc.vector.tensor_copy(
                        dst, htp[:, :].rearrange("p (k b) -> p k b", b=BL)
                    )

                    # --- interleaved projection work ---
                    if t == TCH - 1:
                        emit_gather(0)
                    for grp in proj_sched.get(t, []):
                        emit_proj_group(*grp)

                # ---- tail: second chunk ----
                emit_gather(1)
                for mt in range(NCORES):
                    for vc in range(NV):
                        emit_proj_group(1, mt, vc)

    nc.compile()
    return nc


def _prep_inputs(inputs):
    """Host-side layout prep. Returns per-core input maps."""
    f = lambda x: np.asarray(x, dtype=np.float32)
    targets = np.asarray(inputs["targets"])
    enc_hid = f(inputs["encoder_hidden"])
    enc_hn = f(inputs["enc_hn"])
    enc_cn = f(inputs["enc_cn"])
    emb = f(inputs["emb"])
    ln_enc_g = f(inputs["ln_enc_g"])
    ln_enc_b = f(inputs["ln_enc_b"])
    ln_emb_g = f(inputs["ln_emb_g"])
    ln_emb_b = f(inputs["ln_emb_b"])
    q_w = f(inputs["q_w"])
    q_b = f(inputs["q_b"])
    k_w = f(inputs["k_w"])
    e_w = f(inputs["e_w"])
    w_ih = f(inputs["w_ih"])
    w_hh = f(inputs["w_hh"])
    b_ih = f(inputs["b_ih"])
    b_hh = f(inputs["b_hh"])
    out_w = f(inputs["out_w"])
    out_b = f(inputs["out_b"])

    # h0/c0: tiny NL2-weight linear combos, done on host
    phw = f(inputs["proj_hn_w"])[0]
    phb = float(f(inputs["proj_hn_b"])[0])
    pcw = f(inputs["proj_cn_w"])[0]
    pcb = float(f(inputs["proj_cn_b"])[0])
    h0 = np.einsum("lbh,l->bh", enc_hn, phw) + phb  # [B, H]
    c0 = np.einsum("lbh,l->bh", enc_cn, pcw) + pcb  # [B, H]

    # fold LN affines into adjacent matmuls
    kw_eff = k_w * ln_enc_g[None, :]
    qadd = q_b + k_w @ ln_enc_b
    w_ctx = w_ih[:, :H] * ln_enc_g[None, :]
    w_x = w_ih[:, H:] * ln_emb_g[None, :]
    b_gates = b_ih + b_hh + w_ih[:, :H] @ ln_enc_b + w_ih[:, H:] @ ln_emb_b

    # reorder gate blocks [i, f, g, o] -> [i, f, o, g]
    perm = np.r_[0:H, H : 2 * H, 3 * H : 4 * H, 2 * H : 3 * H]
    w_ctx, w_x, w_hh_p = w_ctx[perm], w_x[perm], w_hh[perm]
    b_gates = b_gates[perm]

    wcT = np.ascontiguousarray(w_ctx.T).astype(bf16)
    whT = np.ascontiguousarray(w_hh_p.T).astype(bf16)
    x_wT = np.concatenate([w_x.T, b_gates[None, :]], axis=0)
    x_wT = np.ascontiguousarray(x_wT).astype(bf16)

    kwT_b = np.ascontiguousarray(kw_eff.T).astype(bf16)
    qwT_b = np.ascontiguousarray(q_w.T).astype(bf16)
    ewT_b = np.ascontiguousarray(e_w[0][:, None]).astype(bf16)
    qadd4 = np.ascontiguousarray(qadd.reshape(KT, 128).T, dtype=np.float32)

    in_maps = []
    for c in range(NCORES):
        bsl = slice(BL * c, BL * (c + 1))
        vs = slice(VL * c, VL * (c + 1))
        enc_c = np.ascontiguousarray(enc_hid[bsl].reshape(BS, H), dtype=np.float32)
        tgt = np.ascontiguousarray(targets[bsl].T.reshape(TB, 1), dtype=np.int32)
        h0T = np.ascontiguousarray(h0[bsl].T).astype(bf16)
        c0_c = np.ascontiguousarray(c0[bsl], dtype=np.float32)
        owT = np.ascontiguousarray(out_w[vs].T).astype(bf16)
        ob = np.ascontiguousarray(np.broadcast_to(out_b[vs].astype(bf16), (128, VL)))
        in_maps.append(
            {
                "enc": enc_c,
                "h0T": h0T,
                "c0": c0_c,
                "emb": emb,
                "tgt": tgt,
                "kwT": kwT_b,
                "qwT": qwT_b,
                "ewT": ewT_b,
                "qadd": qadd4,
                "wcT": wcT,
                "whT": whT,
                "xwT": x_wT,
                "owT": owT,
                "ob": ob,
            }
        )
    return in_maps


_CACHE = {}


def kernel(**inputs) -> np.ndarray:
    in_maps = _prep_inputs(inputs)
    if "nc" not in _CACHE:
        _CACHE["nc"] = build_nc()
    nc = _CACHE["nc"]
    res = run_bass_kernel_spmd(
        nc,
        in_maps,
        core_ids=list(range(NCORES)),
        trace=bool(int(os.environ.get("KERNEL_TRACE", "0"))),
    )
    kernel._last = res
    shards = [res.results[c]["out"] for c in range(NCORES)]
    return np.concatenate(shards, axis=2)


kernel._last = None


if __name__ == "__main__":
    nc = build_nc()
    print("build OK")
